# revision 17
# baseline (speedup 1.0000x reference)
import sys

sys.path.insert(0, "/opt/trn_rl_repo")

import numpy as np

P = 128          # partitions / tile edge
D = 128          # model dim
H = 4            # heads
DH = 32          # head dim
NCORES = 8

# Full-problem geometry (N=100000, E=800000). Each core owns NBLK node
# blocks of 128 nodes; block b's incident edges are padded to TT[b]
# whole 128-edge tiles (TT shared across cores so the SPMD program is
# uniform). All indexing is pre-resolved on the host: x rows are
# duplicated per edge slot and one-hot row-selection matrices ship as
# fp8, so the device does no indirect addressing at all.
NBLK_FULL = 98                      # 98*128 = 12544 own nodes/core
NPAD_FULL = NCORES * NBLK_FULL * P  # 100352 padded nodes
SC = 3                              # tiles per PSUM sub-chunk


def _channel_perm():
    # torch reshape (N, DH, H): flat channel c = d*H + h. We relayout to
    # h-major c' = h*DH + d by permuting weight rows: perm[c'] = d*H + h.
    cp = np.arange(D)
    return (cp % DH) * H + (cp // DH)


def _register_cumsum_op():
    """Fused out = running-sum(in0*in1) along the free stream (f32 out).
    Per-head scores are recovered by differencing the cumsum at
    32-element page ends."""
    from concourse.dve_spec import Spec, Src0, Src1, scan, AluOp, lower
    from concourse.dve_ops import (DveOp, DveOpSpec, OPS, CUSTOM_DVE_SPECS,
                                   _SUB_OPCODE_FOR_NAME, _CUSTOM_DVE_ROW_BASE,
                                   has_src1)
    name = "PROD_CUMSUM_ANT"
    for op in OPS:
        if op.name == name:
            return op

    def _ref(in0, in1, c0, c1, c2):
        p = in0.astype(np.float32) * np.asarray(in1, np.float32)
        sh = p.shape
        return np.cumsum(p.reshape(sh[0], -1), axis=1).reshape(sh)

    spec = Spec(body=scan(AluOp.ADD, Src0 * Src1), reference=_ref)
    _SUB_OPCODE_FOR_NAME[name] = _CUSTOM_DVE_ROW_BASE + len(OPS)
    shas = {}
    for ver in ("v3", "v4"):
        s = DveOpSpec(name=name, opcode=_SUB_OPCODE_FOR_NAME[name],
                      uops=lower(spec, ver=ver), rd1_en=has_src1(spec))
        shas[ver] = s.sha(ver)
    op = DveOp(name, spec, subdim=False, uops_sha=shas)
    OPS.append(op)
    CUSTOM_DVE_SPECS[name] = spec
    return op


def _build_program(NOWN, NBLK, TT):
    import concourse.bass as bass
    import concourse.tile as tile
    from concourse import bacc, mybir
    from concourse.masks import make_identity
    from contextlib import ExitStack

    cumsum_op = _register_cumsum_op()

    dt = mybir.dt
    f32, f16, bf16, f8 = dt.float32, dt.float16, dt.bfloat16, dt.float8e4
    NTt = sum(TT)
    NTS = NTt * P
    toff = np.concatenate([[0], np.cumsum(TT)]).astype(int)

    nc = bacc.Bacc("TRN2", target_bir_lowering=False, debug=False,
                   num_devices=NCORES)

    xot_d = nc.dram_tensor("xot", [D, NOWN], f16, kind="ExternalInput").ap()
    xce_d = nc.dram_tensor("xce", [D, NTS], f16, kind="ExternalInput").ap()
    ss_d = nc.dram_tensor("ss", [P, 2 * NTS], f8, kind="ExternalInput").ap()
    wkv_d = nc.dram_tensor("wkv", [D, 2 * D], f16, kind="ExternalInput").ap()
    wq_d = nc.dram_tensor("wq", [D, D], f16, kind="ExternalInput").ap()
    wo_d = nc.dram_tensor("wo", [D, D], f16, kind="ExternalInput").ap()
    bq_d = nc.dram_tensor("bq", [1, D], f16, kind="ExternalInput").ap()

    out_d = nc.dram_tensor("out", [NOWN, D], f32, kind="ExternalOutput").ap()

    AF = mybir.ActivationFunctionType
    OP = mybir.AluOpType

    with tile.TileContext(nc) as tc, ExitStack() as ctx:
        res = ctx.enter_context(tc.tile_pool(name="res", bufs=1))
        wkv_sb = res.tile([D, 2 * D], f16, name="wkv_sb")
        wq_sb = res.tile([D, D], f16, name="wq_sb")
        wo_sb = res.tile([D, D], f16, name="wo_sb")
        bq_sb = res.tile([1, D], f16, name="bq_sb")
        ones_sb = res.tile([1, P], f16, name="ones_sb")
        ident = res.tile([P, P], f16, name="ident")
        q_all = res.tile([P, NBLK, D], f16, name="q_all")

        for sb_t, dr_t in [(wkv_sb, wkv_d), (wq_sb, wq_d), (wo_sb, wo_d),
                           (bq_sb, bq_d)]:
            nc.sync.dma_start(sb_t[:], dr_t[:])
        nc.vector.memset(ones_sb[:], 1.0)
        make_identity(nc, ident[:])

        # project q (with bias) for all own nodes upfront into SBUF
        CH = 4
        with tc.tile_pool(name="xa", bufs=4) as xa, \
             tc.tile_pool(name="qp", bufs=4, space="PSUM") as qp:
            for j0 in range(0, NBLK, CH):
                c = min(CH, NBLK - j0)
                xo16 = xa.tile([P, c * P], f16, name="xo16")
                nc.sync.dma_start(xo16[:], xot_d[:, j0 * P:(j0 + c) * P])
                for t in range(c):
                    q_ps = qp.tile([P, D], f32, name="q_ps")
                    nc.tensor.matmul(q_ps[:], lhsT=ones_sb[:], rhs=bq_sb[:],
                                     start=True, stop=False)
                    nc.tensor.matmul(q_ps[:],
                                     lhsT=xo16[:, t * P:(t + 1) * P],
                                     rhs=wq_sb[:], start=False, stop=True)
                    if t % 2 == 0:
                        nc.vector.tensor_copy(q_all[:, j0 + t, :], q_ps[:])
                    else:
                        nc.scalar.copy(q_all[:, j0 + t, :], q_ps[:])

        with tc.tile_pool(name="bl", bufs=3) as bl, \
             tc.tile_pool(name="ck", bufs=4) as ck, \
             tc.tile_pool(name="pa", bufs=2, space="PSUM") as pa, \
             tc.tile_pool(name="yp", bufs=2, space="PSUM") as yp:
            for b in range(NBLK):
                nt = TT[b]
                t0 = toff[b]
                s0 = t0 * P
                # block inputs: per-edge source x, combined one-hot selectors
                xc_b = bl.tile([P, nt * P], f16, name="xc_b")
                nc.sync.dma_start(xc_b[:], xce_d[:, s0:s0 + nt * P])
                ss_b = bl.tile([P, 2 * nt * P], f8, name="ss_b")
                nc.sync.dma_start(ss_b[:], ss_d[:, 2 * s0:2 * s0 + 2 * nt * P])
                st_b = ss_b[:, 0:nt * P]
                se_b = ss_b[:, nt * P:2 * nt * P]

                # one PSUM bank per block: ypre | o | yT(f16 view)
                ypk = yp.tile([P, 4 * D], f32, name="ypk")
                ypre = ypk[:, 0:D + H]
                o_ps = ypk[:, D + H:2 * D + H]
                yT_ps = ypk[:, 2 * D + H:2 * D + H + D // 2].bitcast(f16)
                q_sb = q_all[:, b, :]

                k = 0
                pend = None   # deferred scatter: (c0, sc_n, wext)
                for c0 in range(0, nt, SC):
                    sc_n = min(SC, nt - c0)
                    kv_ps = pa.tile([P, SC, 2 * D], f32, name="kv_ps")
                    qx_ps = pa.tile([P, SC, D], f32, name="qx_ps")
                    for i in range(sc_n):
                        t = c0 + i
                        nc.tensor.matmul(
                            qx_ps[:, i, :],
                            lhsT=st_b[:, t * P:(t + 1) * P], rhs=q_sb,
                            start=True, stop=True)
                        nc.tensor.matmul(
                            kv_ps[:, i, :],
                            lhsT=xc_b[:, t * P:(t + 1) * P], rhs=wkv_sb[:],
                            start=True, stop=True)
                    if pend is not None:
                        for (pt, pw) in pend:
                            nc.tensor.matmul(ypre,
                                             lhsT=se_b[:, pt * P:(pt + 1) * P],
                                             rhs=pw,
                                             start=(k == 0),
                                             stop=(k == nt - 1))
                            k += 1
                    qx_sb = ck.tile([P, SC, D], f16, name="qx_sb")
                    nc.scalar.copy(qx_sb[:, 0:sc_n, :], qx_ps[:, 0:sc_n, :])
                    cs = ck.tile([P, SC, D], f32, name="cs")
                    nc.vector._custom_dve(
                        cumsum_op, out=cs[:, 0:sc_n, :],
                        in0=qx_sb[:, 0:sc_n, :],
                        in1=kv_ps[:, 0:sc_n, 0:D])
                    cef = cs[:, 0:sc_n, :].rearrange(
                        "p t (h d) -> p t h d",
                        h=H)[:, :, :, DH - 1:DH].rearrange(
                        "p t h d -> p (t h d)")
                    sc_t = ck.tile([P, SC, H], f32, name="sc_t")
                    scf = sc_t[:, 0:sc_n, :].rearrange("p t h -> p (t h)")
                    nc.gpsimd.tensor_copy(scf[:, 0:1], cef[:, 0:1])
                    nc.gpsimd.tensor_tensor(
                        out=scf[:, 1:], in0=cef[:, 1:],
                        in1=cef[:, 0:sc_n * H - 1], op=OP.subtract)
                    wext = ck.tile([P, SC, D + H], bf16, name="wext")
                    nc.scalar.activation(wext[:, 0:sc_n, D:D + H],
                                         sc_t[:, 0:sc_n, :], AF.Exp)
                    nc.vector.tensor_tensor(
                        out=wext[:, 0:sc_n, 0:D].rearrange(
                            "p t (h d) -> p t h d", h=H),
                        in0=kv_ps[:, 0:sc_n, D:2 * D].rearrange(
                            "p t (h d) -> p t h d", h=H),
                        in1=wext[:, 0:sc_n, D:D + H].to_broadcast(
                            (P, sc_n, H, DH)),
                        op=OP.mult)
                    pend = [(c0 + i, wext[:, i, :]) for i in range(sc_n)]
                for (pt, pw) in pend:
                    nc.tensor.matmul(ypre,
                                     lhsT=se_b[:, pt * P:(pt + 1) * P],
                                     rhs=pw,
                                     start=(k == 0), stop=(k == nt - 1))
                    k += 1

                zr = ck.tile([P, H], f32, name="zr")
                nc.vector.tensor_scalar_add(zr[:], ypre[:, D:D + H], 1e-30)
                rz = ck.tile([P, H], f32, name="rz")
                nc.vector.reciprocal(rz[:], zr[:])
                yb = ck.tile([P, D], f16, name="yb")
                nc.vector.tensor_tensor(
                    out=yb[:].rearrange("p (h d) -> p h d", h=H),
                    in0=ypre[:, 0:D].rearrange("p (h d) -> p h d", h=H),
                    in1=rz[:].to_broadcast((P, H, DH)),
                    op=OP.mult)
                nc.tensor.transpose(yT_ps, yb[:], ident[:])
                yT = ck.tile([P, D], f16, name="yT")
                nc.scalar.copy(yT[:], yT_ps)
                nc.tensor.matmul(o_ps, lhsT=yT[:], rhs=wo_sb[:],
                                 start=True, stop=True)
                o_sb = ck.tile([P, D], f32, name="o_sb")
                nc.scalar.copy(o_sb[:], o_ps)
                nc.scalar.dma_start(out_d[b * P:(b + 1) * P, :], o_sb[:])

    nc.compile()
    return nc


def _plan(row, NOWN, NBLK):
    """Per-block tile counts: max over cores of ceil(edges/128)."""
    row = np.asarray(row, np.int64)
    TT = np.ones(NBLK, np.int64)
    for c in range(NCORES):
        lo, hi = c * NOWN, (c + 1) * NOWN
        e0 = np.searchsorted(row, lo, "left")
        e1 = np.searchsorted(row, hi, "left")
        blk = (row[e0:e1] - lo) // P
        cnts = np.bincount(blk, minlength=NBLK)
        TT = np.maximum(TT, -(-cnts // P))
    return TT.tolist()


def _prepare_inputs(x, row, col, Wq, bq, Wk, bk, Wv, bv, Wo, bo, TT,
                    NOWN, NBLK):
    import ml_dtypes
    f8 = ml_dtypes.float8_e4m3
    N = x.shape[0]
    NPAD = NCORES * NOWN
    perm = _channel_perm()
    s = np.sqrt(float(H))
    wkv_in = np.ascontiguousarray(
        np.concatenate([Wk[perm, :].T, Wv[perm, :].T], axis=1)
    ).astype(np.float16)
    wq_in = np.ascontiguousarray((Wq[perm, :] / s).T).astype(np.float16)
    wo_in = np.ascontiguousarray(Wo[:, perm].T).astype(np.float16)
    bq_in = (bq[perm] / s).reshape(1, D).astype(np.float16)

    x_pad = np.zeros((NPAD, D), np.float32)
    x_pad[:N] = x

    NTt = sum(TT)
    NTS = NTt * P
    toff = np.concatenate([[0], np.cumsum(TT)]).astype(np.int64)
    in_maps = []
    for c in range(NCORES):
        lo, hi = c * NOWN, (c + 1) * NOWN
        e0 = np.searchsorted(row, lo, "left")
        e1 = np.searchsorted(row, hi, "left")
        rows_c = (row[e0:e1] - lo).astype(np.int64)
        cols_c = col[e0:e1].astype(np.int64)
        blk = rows_c // P
        blk_starts = np.searchsorted(blk, np.arange(NBLK), "left")
        rank = np.arange(rows_c.shape[0]) - blk_starts[blk]
        # slot id: block-major tiles, slot i -> (partition i%128, tile i//128)
        slot = toff[blk] * P + rank
        rl = rows_c % P
        xce = np.zeros((NTS, D), np.float16)
        xce[slot] = x_pad[cols_c].astype(np.float16)
        # combined per-block [selt | sel] one-hot stream
        ss = np.zeros((P, 2 * NTS), f8)
        tile_i = slot // P
        part_i = slot % P
        blk_of_tile = np.repeat(np.arange(NBLK), TT)
        # selt: column at 2*toff[b]*P + (local slot)
        sboff = 2 * toff[blk] * P
        loc = slot - toff[blk] * P
        ss[rl, sboff + loc] = 1.0
        # sel: column at 2*toff[b]*P + TT[b]*P + local_tile*P + rl
        TTa = np.asarray(TT, np.int64)
        ss[part_i, sboff + TTa[blk] * P + (tile_i - toff[blk]) * P + rl] = 1.0
        in_maps.append({
            "xot": np.ascontiguousarray(x_pad[lo:hi].T).astype(np.float16),
            "xce": np.ascontiguousarray(xce.T),
            "ss": ss,
            "wkv": wkv_in, "wq": wq_in, "wo": wo_in,
            "bq": bq_in,
        })
    return in_maps


def _install_ntff_hook():
    """The agent image's antenv lacks axon_hooks; inject it so trace=True
    can drive NTFF profiling through libaxon_pjrt.so."""
    import importlib
    try:
        importlib.import_module("antenv.axon_hooks")
        return
    except ImportError:
        pass
    import types
    if "/root/.axon_site" not in sys.path:
        sys.path.insert(0, "/root/.axon_site")
    from trn_agent_boot.trn_boot import _ntff_profile_via_ctypes
    hook = _ntff_profile_via_ctypes("/opt/axon/libaxon_pjrt.so")
    mod = types.ModuleType("antenv.axon_hooks")
    state = {"hook": hook}
    mod.get_axon_ntff_profile_hook = lambda: state["hook"]
    mod.set_axon_ntff_profile_hook = lambda h: state.update(hook=h)
    import antenv
    antenv.axon_hooks = mod
    sys.modules["antenv.axon_hooks"] = mod


def run(x, row, col, Wq, bq, Wk, bk, Wv, bv, Wo, bo, NBLK=NBLK_FULL,
        trace=False, tmpdir=None):
    from concourse import bass_utils
    from concourse.bass_utils import run_bass_kernel_spmd
    if trace:
        _install_ntff_hook()
        bass_utils.upload_artifacts = lambda d: "local://" + d

    x = np.asarray(x, np.float32)
    row = np.asarray(row, np.int64)
    col = np.asarray(col, np.int64)
    N = x.shape[0]
    NOWN = NBLK * P
    assert NCORES * NOWN >= N
    TT = _plan(row, NOWN, NBLK)
    nc = _build_program(NOWN, NBLK, TT)
    in_maps = _prepare_inputs(
        x, row, col,
        np.asarray(Wq, np.float32), np.asarray(bq, np.float32),
        np.asarray(Wk, np.float32), np.asarray(bk, np.float32),
        np.asarray(Wv, np.float32), np.asarray(bv, np.float32),
        np.asarray(Wo, np.float32), np.asarray(bo, np.float32),
        TT, NOWN, NBLK)
    res = run_bass_kernel_spmd(nc, in_maps, list(range(NCORES)), trace=trace,
                               tmpdir=tmpdir)
    out = np.concatenate([res.results[c]["out"] for c in range(NCORES)], 0)
    # bv folds through the output projection exactly (sum_e a_e = 1);
    # the constant output bias is added here instead of on-device.
    bo_full = (np.asarray(bo, np.float32)
               + np.asarray(Wo, np.float32) @ np.asarray(bv, np.float32))
    return (out[:N] + bo_full).astype(np.float32), res


def kernel(**inputs):
    out, _ = run(**inputs)
    return out


# revision 18
# speedup vs baseline: 1.0814x; 1.0814x over previous
import sys

sys.path.insert(0, "/opt/trn_rl_repo")

import numpy as np

P = 128          # partitions / tile edge
D = 128          # model dim
H = 4            # heads
DH = 32          # head dim
NCORES = 8

# Full-problem geometry (N=100000, E=800000). Each core owns NBLK node
# blocks of 128 nodes; block b's incident edges are padded to TT[b]
# whole 128-edge tiles (TT shared across cores so the SPMD program is
# uniform). All indexing is pre-resolved on the host: x rows are
# duplicated per edge slot and one-hot row-selection matrices ship as
# fp8, so the device does no indirect addressing at all.
NBLK_FULL = 98                      # 98*128 = 12544 own nodes/core
NPAD_FULL = NCORES * NBLK_FULL * P  # 100352 padded nodes
SC = 2                              # tiles per PSUM sub-chunk


def _channel_perm():
    # torch reshape (N, DH, H): flat channel c = d*H + h. We relayout to
    # h-major c' = h*DH + d by permuting weight rows: perm[c'] = d*H + h.
    cp = np.arange(D)
    return (cp % DH) * H + (cp // DH)


def _register_cumsum_op():
    """Fused out = running-sum(in0*in1) along the free stream (f32 out).
    Per-head scores are recovered by differencing the cumsum at
    32-element page ends."""
    from concourse.dve_spec import Spec, Src0, Src1, scan, AluOp, lower
    from concourse.dve_ops import (DveOp, DveOpSpec, OPS, CUSTOM_DVE_SPECS,
                                   _SUB_OPCODE_FOR_NAME, _CUSTOM_DVE_ROW_BASE,
                                   has_src1)
    name = "PROD_CUMSUM_ANT"
    for op in OPS:
        if op.name == name:
            return op

    def _ref(in0, in1, c0, c1, c2):
        p = in0.astype(np.float32) * np.asarray(in1, np.float32)
        sh = p.shape
        return np.cumsum(p.reshape(sh[0], -1), axis=1).reshape(sh)

    spec = Spec(body=scan(AluOp.ADD, Src0 * Src1), reference=_ref)
    _SUB_OPCODE_FOR_NAME[name] = _CUSTOM_DVE_ROW_BASE + len(OPS)
    shas = {}
    for ver in ("v3", "v4"):
        s = DveOpSpec(name=name, opcode=_SUB_OPCODE_FOR_NAME[name],
                      uops=lower(spec, ver=ver), rd1_en=has_src1(spec))
        shas[ver] = s.sha(ver)
    op = DveOp(name, spec, subdim=False, uops_sha=shas)
    OPS.append(op)
    CUSTOM_DVE_SPECS[name] = spec
    return op


def _build_program(NOWN, NBLK, TT):
    import concourse.bass as bass
    import concourse.tile as tile
    from concourse import bacc, mybir
    from concourse.masks import make_identity
    from contextlib import ExitStack

    cumsum_op = _register_cumsum_op()

    dt = mybir.dt
    f32, f16, bf16, f8 = dt.float32, dt.float16, dt.bfloat16, dt.float8e4
    NTt = sum(TT)
    NTS = NTt * P
    toff = np.concatenate([[0], np.cumsum(TT)]).astype(int)

    nc = bacc.Bacc("TRN2", target_bir_lowering=False, debug=False,
                   num_devices=NCORES)

    xot_d = nc.dram_tensor("xot", [D, NOWN], f16, kind="ExternalInput").ap()
    xce_d = nc.dram_tensor("xce", [D, NTS], f16, kind="ExternalInput").ap()
    ss_d = nc.dram_tensor("ss", [P, 2 * NTS], f8, kind="ExternalInput").ap()
    wkv_d = nc.dram_tensor("wkv", [D, 2 * D], f16, kind="ExternalInput").ap()
    wq_d = nc.dram_tensor("wq", [D, D], f16, kind="ExternalInput").ap()
    wo_d = nc.dram_tensor("wo", [D, D], f16, kind="ExternalInput").ap()
    bq_d = nc.dram_tensor("bq", [1, D], f16, kind="ExternalInput").ap()

    out_d = nc.dram_tensor("out", [NOWN, D], f32, kind="ExternalOutput").ap()

    AF = mybir.ActivationFunctionType
    OP = mybir.AluOpType

    with tile.TileContext(nc) as tc, ExitStack() as ctx:
        res = ctx.enter_context(tc.tile_pool(name="res", bufs=1))
        wkv_sb = res.tile([D, 2 * D], f16, name="wkv_sb")
        wq_sb = res.tile([D, D], f16, name="wq_sb")
        wo_sb = res.tile([D, D], f16, name="wo_sb")
        bq_sb = res.tile([1, D], f16, name="bq_sb")
        ones_sb = res.tile([1, P], f16, name="ones_sb")
        ident = res.tile([P, P], f16, name="ident")
        q_all = [res.tile([P, D], f16, name=f"q_all{j}")
                 for j in range(NBLK)]

        for sb_t, dr_t in [(wkv_sb, wkv_d), (wq_sb, wq_d), (wo_sb, wo_d),
                           (bq_sb, bq_d)]:
            nc.sync.dma_start(sb_t[:], dr_t[:])
        nc.vector.memset(ones_sb[:], 1.0)
        make_identity(nc, ident[:])

        # project q (with bias) for all own nodes upfront into SBUF
        CH = 4
        with tc.tile_pool(name="xa", bufs=4) as xa, \
             tc.tile_pool(name="qp", bufs=4, space="PSUM") as qp:
            for j0 in range(0, NBLK, CH):
                c = min(CH, NBLK - j0)
                xo16 = xa.tile([P, c * P], f16, name="xo16")
                nc.sync.dma_start(xo16[:], xot_d[:, j0 * P:(j0 + c) * P])
                for t in range(c):
                    q_ps = qp.tile([P, D], f32, name="q_ps")
                    nc.tensor.matmul(q_ps[:], lhsT=ones_sb[:], rhs=bq_sb[:],
                                     start=True, stop=False)
                    nc.tensor.matmul(q_ps[:],
                                     lhsT=xo16[:, t * P:(t + 1) * P],
                                     rhs=wq_sb[:], start=False, stop=True)
                    if t % 2 == 0:
                        nc.vector.tensor_copy(q_all[j0 + t][:], q_ps[:])
                    else:
                        nc.scalar.copy(q_all[j0 + t][:], q_ps[:])

        with tc.tile_pool(name="bl", bufs=3) as bl, \
             tc.tile_pool(name="ck", bufs=4) as ck, \
             tc.tile_pool(name="pa", bufs=3, space="PSUM") as pa, \
             tc.tile_pool(name="yp", bufs=2, space="PSUM") as yp:
            for b in range(NBLK):
                nt = TT[b]
                t0 = toff[b]
                s0 = t0 * P
                # block inputs: per-edge source x, combined one-hot selectors
                xc_b = bl.tile([P, nt * P], f16, name="xc_b")
                nc.sync.dma_start(xc_b[:], xce_d[:, s0:s0 + nt * P])
                ss_b = bl.tile([P, 2 * nt * P], f8, name="ss_b")
                nc.sync.dma_start(ss_b[:], ss_d[:, 2 * s0:2 * s0 + 2 * nt * P])
                st_b = ss_b[:, 0:nt * P]
                se_b = ss_b[:, nt * P:2 * nt * P]

                # one PSUM bank per block: ypre | o | yT(f16 view)
                ypk = yp.tile([P, 4 * D], f32, name="ypk")
                ypre = ypk[:, 0:D + H]
                o_ps = ypk[:, D + H:2 * D + H]
                yT_ps = ypk[:, 2 * D + H:2 * D + H + D // 2].bitcast(f16)
                q_sb = q_all[b][:]

                k = 0
                pend = None   # deferred scatter: (c0, sc_n, wext)
                for c0 in range(0, nt, SC):
                    sc_n = min(SC, nt - c0)
                    kv_ps = pa.tile([P, SC, 2 * D], f32, name="kv_ps")
                    qx_ps = pa.tile([P, SC, D], f32, name="qx_ps")
                    for i in range(sc_n):
                        t = c0 + i
                        nc.tensor.matmul(
                            qx_ps[:, i, :],
                            lhsT=st_b[:, t * P:(t + 1) * P], rhs=q_sb,
                            start=True, stop=True)
                        nc.tensor.matmul(
                            kv_ps[:, i, :],
                            lhsT=xc_b[:, t * P:(t + 1) * P], rhs=wkv_sb[:],
                            start=True, stop=True)
                    if pend is not None:
                        for (pt, pw) in pend:
                            nc.tensor.matmul(ypre,
                                             lhsT=se_b[:, pt * P:(pt + 1) * P],
                                             rhs=pw,
                                             start=(k == 0),
                                             stop=(k == nt - 1))
                            k += 1
                    qx_sb = ck.tile([P, SC, D], f16, name="qx_sb")
                    nc.scalar.copy(qx_sb[:, 0:sc_n, :], qx_ps[:, 0:sc_n, :])
                    cs = ck.tile([P, SC, D], f32, name="cs")
                    nc.vector._custom_dve(
                        cumsum_op, out=cs[:, 0:sc_n, :],
                        in0=qx_sb[:, 0:sc_n, :],
                        in1=kv_ps[:, 0:sc_n, 0:D])
                    cef = cs[:, 0:sc_n, :].rearrange(
                        "p t (h d) -> p t h d",
                        h=H)[:, :, :, DH - 1:DH].rearrange(
                        "p t h d -> p (t h d)")
                    sc_t = ck.tile([P, SC, H], f32, name="sc_t")
                    scf = sc_t[:, 0:sc_n, :].rearrange("p t h -> p (t h)")
                    nc.gpsimd.tensor_copy(scf[:, 0:1], cef[:, 0:1])
                    nc.gpsimd.tensor_tensor(
                        out=scf[:, 1:], in0=cef[:, 1:],
                        in1=cef[:, 0:sc_n * H - 1], op=OP.subtract)
                    wext = ck.tile([P, SC, D + H], bf16, name="wext")
                    nc.scalar.activation(wext[:, 0:sc_n, D:D + H],
                                         sc_t[:, 0:sc_n, :], AF.Exp)
                    nc.vector.tensor_tensor(
                        out=wext[:, 0:sc_n, 0:D].rearrange(
                            "p t (h d) -> p t h d", h=H),
                        in0=kv_ps[:, 0:sc_n, D:2 * D].rearrange(
                            "p t (h d) -> p t h d", h=H),
                        in1=wext[:, 0:sc_n, D:D + H].to_broadcast(
                            (P, sc_n, H, DH)),
                        op=OP.mult)
                    pend = [(c0 + i, wext[:, i, :]) for i in range(sc_n)]
                for (pt, pw) in pend:
                    nc.tensor.matmul(ypre,
                                     lhsT=se_b[:, pt * P:(pt + 1) * P],
                                     rhs=pw,
                                     start=(k == 0), stop=(k == nt - 1))
                    k += 1

                zr = ck.tile([P, H], f32, name="zr")
                nc.vector.tensor_scalar_add(zr[:], ypre[:, D:D + H], 1e-30)
                rz = ck.tile([P, H], f32, name="rz")
                nc.vector.reciprocal(rz[:], zr[:])
                yb = ck.tile([P, D], f16, name="yb")
                nc.vector.tensor_tensor(
                    out=yb[:].rearrange("p (h d) -> p h d", h=H),
                    in0=ypre[:, 0:D].rearrange("p (h d) -> p h d", h=H),
                    in1=rz[:].to_broadcast((P, H, DH)),
                    op=OP.mult)
                nc.tensor.transpose(yT_ps, yb[:], ident[:])
                yT = ck.tile([P, D], f16, name="yT")
                nc.scalar.copy(yT[:], yT_ps)
                nc.tensor.matmul(o_ps, lhsT=yT[:], rhs=wo_sb[:],
                                 start=True, stop=True)
                o_sb = ck.tile([P, D], f32, name="o_sb")
                nc.scalar.copy(o_sb[:], o_ps)
                nc.scalar.dma_start(out_d[b * P:(b + 1) * P, :], o_sb[:])

    nc.compile()
    return nc


def _plan(row, NOWN, NBLK):
    """Per-block tile counts: max over cores of ceil(edges/128)."""
    row = np.asarray(row, np.int64)
    TT = np.ones(NBLK, np.int64)
    for c in range(NCORES):
        lo, hi = c * NOWN, (c + 1) * NOWN
        e0 = np.searchsorted(row, lo, "left")
        e1 = np.searchsorted(row, hi, "left")
        blk = (row[e0:e1] - lo) // P
        cnts = np.bincount(blk, minlength=NBLK)
        TT = np.maximum(TT, -(-cnts // P))
    return TT.tolist()


def _prepare_inputs(x, row, col, Wq, bq, Wk, bk, Wv, bv, Wo, bo, TT,
                    NOWN, NBLK):
    import ml_dtypes
    f8 = ml_dtypes.float8_e4m3
    N = x.shape[0]
    NPAD = NCORES * NOWN
    perm = _channel_perm()
    s = np.sqrt(float(H))
    wkv_in = np.ascontiguousarray(
        np.concatenate([Wk[perm, :].T, Wv[perm, :].T], axis=1)
    ).astype(np.float16)
    wq_in = np.ascontiguousarray((Wq[perm, :] / s).T).astype(np.float16)
    wo_in = np.ascontiguousarray(Wo[:, perm].T).astype(np.float16)
    bq_in = (bq[perm] / s).reshape(1, D).astype(np.float16)

    x_pad = np.zeros((NPAD, D), np.float32)
    x_pad[:N] = x

    NTt = sum(TT)
    NTS = NTt * P
    toff = np.concatenate([[0], np.cumsum(TT)]).astype(np.int64)
    in_maps = []
    for c in range(NCORES):
        lo, hi = c * NOWN, (c + 1) * NOWN
        e0 = np.searchsorted(row, lo, "left")
        e1 = np.searchsorted(row, hi, "left")
        rows_c = (row[e0:e1] - lo).astype(np.int64)
        cols_c = col[e0:e1].astype(np.int64)
        blk = rows_c // P
        blk_starts = np.searchsorted(blk, np.arange(NBLK), "left")
        rank = np.arange(rows_c.shape[0]) - blk_starts[blk]
        # slot id: block-major tiles, slot i -> (partition i%128, tile i//128)
        slot = toff[blk] * P + rank
        rl = rows_c % P
        xce = np.zeros((NTS, D), np.float16)
        xce[slot] = x_pad[cols_c].astype(np.float16)
        # combined per-block [selt | sel] one-hot stream
        ss = np.zeros((P, 2 * NTS), f8)
        tile_i = slot // P
        part_i = slot % P
        blk_of_tile = np.repeat(np.arange(NBLK), TT)
        # selt: column at 2*toff[b]*P + (local slot)
        sboff = 2 * toff[blk] * P
        loc = slot - toff[blk] * P
        ss[rl, sboff + loc] = 1.0
        # sel: column at 2*toff[b]*P + TT[b]*P + local_tile*P + rl
        TTa = np.asarray(TT, np.int64)
        ss[part_i, sboff + TTa[blk] * P + (tile_i - toff[blk]) * P + rl] = 1.0
        in_maps.append({
            "xot": np.ascontiguousarray(x_pad[lo:hi].T).astype(np.float16),
            "xce": np.ascontiguousarray(xce.T),
            "ss": ss,
            "wkv": wkv_in, "wq": wq_in, "wo": wo_in,
            "bq": bq_in,
        })
    return in_maps


def _install_ntff_hook():
    """The agent image's antenv lacks axon_hooks; inject it so trace=True
    can drive NTFF profiling through libaxon_pjrt.so."""
    import importlib
    try:
        importlib.import_module("antenv.axon_hooks")
        return
    except ImportError:
        pass
    import types
    if "/root/.axon_site" not in sys.path:
        sys.path.insert(0, "/root/.axon_site")
    from trn_agent_boot.trn_boot import _ntff_profile_via_ctypes
    hook = _ntff_profile_via_ctypes("/opt/axon/libaxon_pjrt.so")
    mod = types.ModuleType("antenv.axon_hooks")
    state = {"hook": hook}
    mod.get_axon_ntff_profile_hook = lambda: state["hook"]
    mod.set_axon_ntff_profile_hook = lambda h: state.update(hook=h)
    import antenv
    antenv.axon_hooks = mod
    sys.modules["antenv.axon_hooks"] = mod


def run(x, row, col, Wq, bq, Wk, bk, Wv, bv, Wo, bo, NBLK=NBLK_FULL,
        trace=False, tmpdir=None):
    from concourse import bass_utils
    from concourse.bass_utils import run_bass_kernel_spmd
    if trace:
        _install_ntff_hook()
        bass_utils.upload_artifacts = lambda d: "local://" + d

    x = np.asarray(x, np.float32)
    row = np.asarray(row, np.int64)
    col = np.asarray(col, np.int64)
    N = x.shape[0]
    NOWN = NBLK * P
    assert NCORES * NOWN >= N
    TT = _plan(row, NOWN, NBLK)
    nc = _build_program(NOWN, NBLK, TT)
    in_maps = _prepare_inputs(
        x, row, col,
        np.asarray(Wq, np.float32), np.asarray(bq, np.float32),
        np.asarray(Wk, np.float32), np.asarray(bk, np.float32),
        np.asarray(Wv, np.float32), np.asarray(bv, np.float32),
        np.asarray(Wo, np.float32), np.asarray(bo, np.float32),
        TT, NOWN, NBLK)
    res = run_bass_kernel_spmd(nc, in_maps, list(range(NCORES)), trace=trace,
                               tmpdir=tmpdir)
    out = np.concatenate([res.results[c]["out"] for c in range(NCORES)], 0)
    # bv folds through the output projection exactly (sum_e a_e = 1);
    # the constant output bias is added here instead of on-device.
    bo_full = (np.asarray(bo, np.float32)
               + np.asarray(Wo, np.float32) @ np.asarray(bv, np.float32))
    return (out[:N] + bo_full).astype(np.float32), res


def kernel(**inputs):
    out, _ = run(**inputs)
    return out
